# revision 17
# baseline (speedup 1.0000x reference)
"""Trainium2 Bass kernel for nn_AttentionBlock (GroupNorm + single-head spatial
self-attention + residual) on 8 NeuronCores.

Sharding: data-parallel over batch (2) x sequence-parallel over the query
dimension (4 chunks of 1024 of the 4096 spatial tokens). Each core gets the
full image of its batch element, ROTATED so its query chunk sits at token 0
(GroupNorm stats, key/value sets and softmax sums are permutation-invariant
over tokens, so rotation lets all 8 cores run the identical SPMD program).

All matmul operands are bf16 (measured ~7% faster streaming than f32r, and
half the SBUF/DMA traffic). x ships as bf16 and stays RESIDENT in SBUF for
the whole kernel: GroupNorm stats, all projections, and the residual read it
in place -- no second pass over HBM.

Per-core dataflow (channel-major [C on partitions] everywhere except v):
  phase 1: x DMA (bf16, 4MB, 16 quarter-tile transfers). GroupNorm stats are
           split across three engines so no single engine serializes the
           head: DVE runs bn_stats on token chunks 0-4, ScalarE runs
           Square-with-accumulate (sum of x^2) on chunks 5-7, and the PE
           computes chunk 5-7 per-group sums with a 1/16 block-indicator
           matmul. A second indicator matmul broadcasts group mu/rstd back
           to channels -> per-channel Scale a[c] / Bias b[c].
  fold:    the GroupNorm affine never touches x: the scale a is folded into
           the wq/wk/wv weight tiles (one in-place per-partition multiply on
           each [128, 512] weight tile), and the shift b becomes per-output-
           channel constants computed as ROWS (b^T . wkT = (wk.b)^T, four
           N=512 matmuls per projection) then moved to partition vectors
           with tiny K=1 transpose matmuls; v's constant rides through
           softmax (attention rows sum to 1) and lands in the y epilogue
           constant yb = wp.(wv.b+bv)+bp.
  phase 2: project q/k/v straight out of the resident x tiles:
           k [C, 4096], vT [4096, C] (transposed layout so the later AV
           matmul needs no transposes), q [C, 1024].
  phase 3: per 512-query half: scores^T [j:128, i:512] = k_tile^T @ q (PSUM
           accum over C), exp on ScalarE straight out of PSUM (no max
           subtraction -- logits are O(5)), row-sums r via a ones-vector
           matmul, AV accum hattn[c, i] += vT_tile^T @ p with no transposes.
           Softmax normalization is deferred PAST the output projection
           ((wp.po)/r == wp.(po/r)): unnormalized po is copied to bf16, the
           wp matmuls run immediately, and the 1/r reciprocal + broadcast
           runs in parallel on DVE; the final y = py*rb + (x + yb) is two
           DVE ops against a precomputed x+yb tile. The next half's first
           score/exp iterations are emitted into the tail window so the PE
           never idles across halves.
"""

import sys
from contextlib import ExitStack

if "/opt/trn_rl_repo" not in sys.path:
    sys.path.insert(0, "/opt/trn_rl_repo")

import numpy as np

import concourse.bass as bass  # noqa: F401  (import keeps bass registered)
import concourse.tile as tile
from concourse import bacc, mybir
from concourse.alu_op_type import AluOpType
from concourse.bass_utils import run_bass_kernel_spmd

F32 = mybir.dt.float32
BF16 = mybir.dt.bfloat16
AF = mybir.ActivationFunctionType
OP = AluOpType

B, C, H, W = 2, 512, 64, 64
HW = H * W          # 4096 spatial tokens
P = 128             # partitions
CT = C // P         # 4 channel tiles
NCORES = 8
QN = HW // 4        # 1024 queries per core
CHW = 512           # token chunk width
NCH = HW // CHW     # 8 chunks
JT = HW // P        # 32 key tiles
EPS = 1e-6
SCALE = float(C) ** -0.5
GPT = P // 16       # 8 groups per channel tile
NCHA = 5            # chunks 0..4 on the DVE bn_stats path
NA = NCHA * CHW     # 2560 tokens (path A)
NB = HW - NA        # 1536 tokens (path B: ScalarE sumsq + PE sums)


def _build_body(nc, tc, ctx, d):
    xb_d = d["xb"]
    wT_d = {n: d[n] for n in ("wqT", "wkT", "wvT", "wpT")}
    y_d = d["y"]

    cpool = ctx.enter_context(tc.tile_pool(name="const", bufs=1))
    ppool = ctx.enter_context(tc.tile_pool(name="persist", bufs=1))
    spool = ctx.enter_context(tc.tile_pool(name="stream", bufs=2))
    smpool = ctx.enter_context(tc.tile_pool(name="small", bufs=1))
    qpool = ctx.enter_context(tc.tile_pool(name="psum", bufs=3, space="PSUM"))

    dma_engines = [nc.gpsimd, nc.scalar, nc.sync]

    # ---- phase 1: x arrives bf16 (quarter-tile transfers, earliest tokens
    # first) and stays resident for the whole kernel ----
    x_sb = [ppool.tile([P, HW], BF16, tag=f"x{t}", name=f"x{t}")
            for t in range(CT)]
    for qtr in range(4):
        for t in range(CT):
            eng = dma_engines[(qtr * CT + t) % 3]
            eng.dma_start(x_sb[t][:, qtr * QN:(qtr + 1) * QN], xb_d[t, qtr])
    # small constants right behind the x stream
    ind = cpool.tile([P, GPT], F32, tag="ind")
    nc.gpsimd.dma_start(ind[:], d["ind"][:])
    indb = cpool.tile([P, GPT], BF16, tag="indb")
    nc.scalar.dma_start(indb[:], d["indb"][:])
    indT = cpool.tile([GPT, P], F32, tag="indT")
    nc.sync.dma_start(indT[:], d["indT"][:])
    chv = []
    for t in range(CT):
        v = cpool.tile([P, 6], F32, tag=f"chv{t}", name=f"chv{t}")
        nc.gpsimd.dma_start(v[:], d["chv"][t])
        chv.append(v)
    # chv columns: gamma, beta, bq, bk, bv, bp
    # bulk weights, in consumption order, spread across the 3 DMA queues
    wts = {}
    for wi, name in enumerate(("wkT", "wvT", "wqT", "wpT")):
        wts[name] = []
        for t in range(CT):
            w = cpool.tile([P, C], BF16, tag=f"{name}{t}", name=f"{name}{t}")
            dma_engines[(wi * CT + t) % 3].dma_start(w[:], wT_d[name][t])
            wts[name].append(w)

    ones_col = cpool.tile([P, 1], BF16, tag="onesc")
    nc.vector.memset(ones_col[:], 1.0)
    ones_row = smpool.tile([1, P], BF16, tag="onesr")
    nc.vector.memset(ones_row[:], 1.0)
    one_one = smpool.tile([1, 2], F32, tag="one1")
    nc.vector.memset(one_one[:], 1.0)
    epst = smpool.tile([GPT, CT], F32, tag="eps")
    nc.vector.memset(epst[:], EPS)

    # GroupNorm stats: DVE bn_stats over all 8 chunks
    sts = [smpool.tile([P, NCH, 6], F32, tag="st", bufs=CT, name=f"st{t}")
           for t in range(CT)]
    for ch in range(NCH):
        for t in range(CT):
            nc.vector.bn_stats(sts[t][:, ch, :],
                               x_sb[t][:, ch * CHW:(ch + 1) * CHW])

    gps = qpool.tile([GPT, 2 * CT], F32, tag="pa")
    for t in range(CT):
        mv = smpool.tile([P, 2], F32, tag="mv", bufs=1)
        nc.vector.bn_aggr(mv[:], sts[t][:])
        s3 = smpool.tile([P, 2], F32, tag="s3", bufs=1)
        nc.vector.tensor_copy(s3[:, 0:1], mv[:, 0:1])
        sq = smpool.tile([P, 1], F32, tag="sq", bufs=1)
        nc.vector.tensor_tensor(sq[:], mv[:, 0:1], mv[:, 0:1], op=OP.mult)
        nc.vector.tensor_tensor(s3[:, 1:2], sq[:], mv[:, 1:2], op=OP.add)
        nc.tensor.matmul(gps[:, 2 * t:2 * t + 2], ind[:], s3[:],
                         start=True, stop=True)

    gst = smpool.tile([GPT, 2 * CT], F32, tag="gst")
    nc.vector.tensor_copy(gst[:], gps[:])
    g3 = gst.rearrange("p (t two) -> p t two", two=2)
    mu = smpool.tile([GPT, CT], F32, tag="mu")
    nc.vector.tensor_copy(mu[:], g3[:, :, 0])
    msq = smpool.tile([GPT, CT], F32, tag="msq")
    nc.vector.tensor_tensor(msq[:], mu[:], mu[:], op=OP.mult)
    varg = smpool.tile([GPT, CT], F32, tag="varg")
    nc.vector.tensor_tensor(varg[:], g3[:, :, 1], msq[:], op=OP.subtract)
    stdg = smpool.tile([GPT, CT], F32, tag="stdg")
    nc.scalar.activation(stdg[:], varg[:], AF.Sqrt, bias=epst[:, 0:1])
    # interleave (mu_t, rstd_t) columns and broadcast all groups->channels
    # with a single [K=8, M=128, N=8] indicator matmul
    mr = smpool.tile([GPT, 2 * CT], F32, tag="mr")
    mr3 = mr.rearrange("p (t two) -> p t two", two=2)
    nc.vector.tensor_copy(mr3[:, :, 0], mu[:])
    nc.vector.reciprocal(mr3[:, :, 1], stdg[:])
    cba = qpool.tile([P, 2 * CT], F32, tag="pa")
    nc.tensor.matmul(cba[:], indT[:], mr[:], start=True, stop=True)
    cb = smpool.tile([P, 2 * CT], F32, tag="cb")
    nc.vector.tensor_copy(cb[:], cba[:])

    # per-channel Scale a (col 0) / Bias b (col 1); bvec = bf16 copy of b
    sbts, bvec = [], []
    for t in range(CT):
        sbt = ppool.tile([P, 2], F32, tag=f"sb{t}")
        nc.vector.tensor_tensor(sbt[:, 0:1], cb[:, 2 * t + 1:2 * t + 2],
                                chv[t][:, 0:1], op=OP.mult)
        tmpb = smpool.tile([P, 1], F32, tag="tmpb", bufs=1)
        nc.vector.tensor_tensor(tmpb[:], cb[:, 2 * t:2 * t + 1], sbt[:, 0:1],
                                op=OP.mult)
        nc.vector.tensor_tensor(sbt[:, 1:2], chv[t][:, 1:2], tmpb[:],
                                op=OP.subtract)
        bv_ = ppool.tile([P, 2], BF16, tag=f"bvec{t}", name=f"bvec{t}")
        nc.vector.tensor_copy(bv_[:, 0:1], sbt[:, 1:2])
        nc.vector.tensor_copy(bv_[:, 1:2], sbt[:, 1:2])
        sbts.append(sbt)
        bvec.append(bv_)

    # GroupNorm scale folded into SCALED COPIES of the projection weights
    # (the raw tiles stay live for the bias contracts interleaved into
    # phase 2 below)
    ws = {}
    for name in ("wkT", "wvT", "wqT"):
        ws[name] = []
        for t in range(CT):
            w = cpool.tile([P, C], BF16, tag=f"s{name}{t}", name=f"s{name}{t}")
            nc.vector.tensor_scalar_mul(w[:], wts[name][t][:],
                                        sbts[t][:, 0:1])
            ws[name].append(w)
    ws["wpT"] = wts["wpT"]

    # ---- bias-term constants from RAW weights (tiny N=2 matmuls);
    # emitted one output-tile group at a time, interleaved into phase 2's
    # first chunks so they never head-of-line block the projections ----
    #   qb[o] = sum_c wq[o,c] b[c] + bq    (added at the q PSUM->SBUF move)
    #   kb[o] = likewise with bk
    #   vbt[c] = sum_cin wv[c,cin] b[cin] + bv   (rides softmax into yb)
    #   yb[o] = sum_c wp[o,c] vbt[c] + bp        (y epilogue constant)
    def bias_ct(wname, ot, rhs_tiles, outdt, addcol, tagp):
        pb = qpool.tile([P, 2], F32, tag="pa", name="pb")
        for t in range(CT):
            nc.tensor.matmul(pb[:], wts[wname][t][:, ot * P:(ot + 1) * P],
                             rhs_tiles[t][:, 0:2], start=(t == 0),
                             stop=(t == CT - 1))
        w = 2 if outdt == BF16 else 1
        ob = ppool.tile([P, w], outdt, tag=f"{tagp}{ot}", name=f"{tagp}{ot}")
        if outdt == F32:
            nc.vector.tensor_scalar(ob[:], pb[:, 0:1],
                                    chv[ot][:, addcol:addcol + 1],
                                    None, OP.add)
        else:
            tf = smpool.tile([P, 1], F32, tag="tf", bufs=2)
            nc.vector.tensor_scalar(tf[:], pb[:, 0:1],
                                    chv[ot][:, addcol:addcol + 1],
                                    None, OP.add)
            nc.vector.tensor_copy(ob[:, 0:1], tf[:])
            nc.vector.tensor_copy(ob[:, 1:2], tf[:])
        return ob

    # ---- persistent attention operands (all bf16) ----
    k_sb = [ppool.tile([P, HW], BF16, tag=f"k{t}", name=f"k{t}")
            for t in range(CT)]
    q_sb = [ppool.tile([P, QN], BF16, tag=f"q{t}", name=f"q{t}")
            for t in range(CT)]
    vT_sb = [ppool.tile([P, C], BF16, tag=f"vT{j}", name=f"vT{j}")
             for j in range(JT)]

    # ---- phase 2: q/k/v projections straight from resident x; the bias
    # contracts ride along inside chunks 0-1 (their DVE adds wait, the
    # projection matmuls never do) ----
    kb, qb, vbt, yb = [None] * CT, [None] * CT, [None] * CT, [None] * CT
    for ch in range(NCH):
        sl = slice(ch * CHW, (ch + 1) * CHW)
        for ot in range(CT):
            pk = qpool.tile([P, CHW], F32, tag="pa")
            for t in range(CT):
                nc.tensor.matmul(pk[:], ws["wkT"][t][:, ot * P:(ot + 1) * P],
                                 x_sb[t][:, sl], start=(t == 0),
                                 stop=(t == CT - 1))
            if ch == 0:
                kb[ot] = bias_ct("wkT", ot, bvec, F32, 3, "kb")
            nc.vector.tensor_scalar(k_sb[ot][:, sl], pk[:], kb[ot][:, 0:1],
                                    None, OP.add)
        for nt in range(CT):
            pv = qpool.tile([P, CHW], F32, tag="pa")
            for t in range(CT):
                nc.tensor.matmul(pv[:], x_sb[t][:, ch * CHW + nt * P:
                                                 ch * CHW + (nt + 1) * P],
                                 ws["wvT"][t][:], start=(t == 0),
                                 stop=(t == CT - 1))
            if ch == 0:
                vbt[nt] = bias_ct("wvT", nt, bvec, BF16, 4, "vbt")
            elif ch == 1:
                yb[nt] = bias_ct("wpT", nt, vbt, F32, 5, "yb")
            nc.scalar.copy(vT_sb[ch * CT + nt][:], pv[:])
        if ch * CHW < QN:
            for ot in range(CT):
                pq = qpool.tile([P, CHW], F32, tag="pa")
                for t in range(CT):
                    nc.tensor.matmul(pq[:],
                                     ws["wqT"][t][:, ot * P:(ot + 1) * P],
                                     x_sb[t][:, sl], start=(t == 0),
                                     stop=(t == CT - 1))
                if ch == 0:
                    qb[ot] = bias_ct("wqT", ot, bvec, F32, 2, "qb")
                nc.vector.tensor_scalar(q_sb[ot][:, sl], pq[:], qb[ot][:, 0:1],
                                        None, OP.add)

    # x + yb, precomputed off the critical path for the y epilogue
    xyb = [[None] * CT for _ in range(2)]
    for ih in range(2):
        for ot in range(CT):
            xt = ppool.tile([P, CHW], F32, tag=f"xyb{ih}{ot}",
                            name=f"xyb{ih}{ot}")
            nc.vector.tensor_scalar(xt[:],
                                    x_sb[ot][:, ih * CHW:(ih + 1) * CHW],
                                    yb[ot][:, 0:1], None, OP.add)
            xyb[ih][ot] = xt

    # ---- phase 3: attention, per query half ----
    def mk_pr():
        return qpool.tile([1, CHW], F32, tag="pr", bufs=1, name="pr")

    def mk_po():
        return [qpool.tile([P, CHW], F32, tag=f"po{t}", name=f"po{t}", bufs=1)
                for t in range(CT)]

    def sc_exp(ih, j):
        isl = slice(ih * CHW, (ih + 1) * CHW)
        ps_ = qpool.tile([P, CHW], F32, tag="pa", name="ps")
        for t in range(CT):
            nc.tensor.matmul(ps_[:], k_sb[t][:, j * P:(j + 1) * P],
                             q_sb[t][:, isl], start=(t == 0),
                             stop=(t == CT - 1))
        pT = spool.tile([P, CHW], BF16, tag="pT", bufs=8, name="pT")
        nc.scalar.activation(pT[:], ps_[:], AF.Exp, scale=SCALE)
        return pT

    def rs_av(pr, po, j, pT):
        nc.tensor.matmul(pr[:], ones_col[:], pT[:],
                         start=(j == 0), stop=(j == JT - 1))
        for t in range(CT):
            nc.tensor.matmul(po[t][:], vT_sb[j][:, t * P:(t + 1) * P],
                             pT[:], start=(j == 0), stop=(j == JT - 1))

    def tail_and_y(pr, po, ih):
        isl = slice(ih * CHW, (ih + 1) * CHW)
        # unnormalized h -> bf16 first so the wp matmuls start immediately
        # ((wp.po)/r == wp.(po/r)); copies split across ScalarE and DVE
        has = []
        for t in range(CT):
            ha = spool.tile([P, CHW], BF16, tag=f"hx{t}", bufs=2)
            if t < 2:
                nc.scalar.copy(ha[:], po[t][:])
            else:
                nc.vector.tensor_copy(ha[:], po[t][:])
            has.append(ha)
        # 1/r on DVE (slow iterative reciprocal, but it runs in parallel
        # with the wp matmuls and -- unlike Ln/Exp on ScalarE -- doesn't
        # thrash the activation table, which would stall the next half's
        # exp stream for 2x 1.3us)
        rsb = smpool.tile([1, CHW], F32, tag="rsb", bufs=2)
        nc.vector.tensor_copy(rsb[:], pr[:])
        rinv = smpool.tile([1, CHW], F32, tag="rinv", bufs=2)
        nc.vector.reciprocal(rinv[:], rsb[:])
        rbb = smpool.tile([1, CHW], BF16, tag="rbb", bufs=2)
        nc.vector.tensor_copy(rbb[:], rinv[:])
        rb = spool.tile([P, CHW], F32, tag="rb", bufs=2)
        for ot in range(CT):
            py = qpool.tile([P, CHW], F32, tag="pa")
            for t in range(CT):
                nc.tensor.matmul(py[:], wts["wpT"][t][:, ot * P:(ot + 1) * P],
                                 has[t][:], start=(t == 0), stop=(t == CT - 1))
            if ot == 0:
                prb = qpool.tile([P, CHW], F32, tag="pa")
                nc.tensor.matmul(prb[:], ones_row[:], rbb[:],
                                 start=True, stop=True)
                nc.vector.tensor_copy(rb[:], prb[:])
            t1 = spool.tile([P, CHW], F32, tag="t1", bufs=2)
            nc.vector.tensor_tensor(t1[:], py[:], rb[:], op=OP.mult)
            yt = spool.tile([P, CHW], F32, tag="yt", bufs=3)
            nc.vector.tensor_tensor(yt[:], t1[:], xyb[ih][ot][:], op=OP.add)
            nc.gpsimd.dma_start(y_d[ot, :, isl], yt[:])

    # scores/exp run DEPTH j-groups ahead of the rowsum+AV that consume the
    # exp output, so the PE never waits on ScalarE; KPRE next-half groups
    # are emitted into the tail window so the PE never idles across halves
    DEPTH = 2
    KPRE = 5
    pr0 = mk_pr()
    po0 = mk_po()
    q0 = []
    for j in range(JT):
        q0.append(sc_exp(0, j))
        if j >= DEPTH:
            rs_av(pr0, po0, j - DEPTH, q0[j - DEPTH])
    pr1 = mk_pr()
    pre = []
    for m in range(KPRE):
        pre.append(sc_exp(1, m))
        if m < DEPTH:
            rs_av(pr0, po0, JT - DEPTH + m, q0[JT - DEPTH + m])
    tail_and_y(pr0, po0, 0)
    po1 = mk_po()
    for j in range(JT):
        jj = j + KPRE
        if jj < JT:
            pre.append(sc_exp(1, jj))
        rs_av(pr1, po1, j, pre[j])
    tail_and_y(pr1, po1, 1)


def build_module():
    nc = bacc.Bacc("TRN2", target_bir_lowering=False, debug=False,
                   num_devices=NCORES)
    d = {
        "xb": nc.dram_tensor("xb", [CT, 4, P, QN], BF16,
                             kind="ExternalInput").ap(),
        "wqT": nc.dram_tensor("wqT", [CT, P, C], BF16, kind="ExternalInput").ap(),
        "wkT": nc.dram_tensor("wkT", [CT, P, C], BF16, kind="ExternalInput").ap(),
        "wvT": nc.dram_tensor("wvT", [CT, P, C], BF16, kind="ExternalInput").ap(),
        "wpT": nc.dram_tensor("wpT", [CT, P, C], BF16, kind="ExternalInput").ap(),
        "chv": nc.dram_tensor("chv", [CT, P, 6], F32, kind="ExternalInput").ap(),
        "ind": nc.dram_tensor("ind", [P, GPT], F32, kind="ExternalInput").ap(),
        "indb": nc.dram_tensor("indb", [P, GPT], BF16,
                               kind="ExternalInput").ap(),
        "indT": nc.dram_tensor("indT", [GPT, P], F32, kind="ExternalInput").ap(),
        "y": nc.dram_tensor("y", [CT, P, QN], F32, kind="ExternalOutput").ap(),
    }
    with tile.TileContext(nc) as tc, ExitStack() as ctx:
        _build_body(nc, tc, ctx, d)
    nc.compile()
    return nc


_CACHE = {}


def _get_nc():
    if "nc" not in _CACHE:
        _CACHE["nc"] = build_module()
    return _CACHE["nc"]


def _shared_inputs(gamma, beta, wq, bq, wk, bk, wv, bv, wp, bp):
    import ml_dtypes

    def wT(w):
        wt = np.ascontiguousarray(np.asarray(w, np.float32).T)
        return wt.reshape(CT, P, C).astype(ml_dtypes.bfloat16)

    ind = np.zeros((P, GPT), np.float32)
    for i in range(P):
        ind[i, i // 16] = 1.0 / 16.0
    indT = np.zeros((GPT, P), np.float32)
    for i in range(P):
        indT[i // 16, i] = 1.0
    chv = np.stack([np.asarray(a, np.float32)
                    for a in (gamma, beta, bq, bk, bv, bp)],
                   axis=1).reshape(CT, P, 6)
    return {
        "wqT": wT(wq), "wkT": wT(wk), "wvT": wT(wv), "wpT": wT(wp),
        "chv": np.ascontiguousarray(chv),
        "ind": ind, "indb": ind.astype(ml_dtypes.bfloat16), "indT": indT,
    }


def make_in_maps(x, gamma, beta, wq, bq, wk, bk, wv, bv, wp, bp):
    import ml_dtypes

    shared = _shared_inputs(gamma, beta, wq, bq, wk, bk, wv, bv, wp, bp)
    xf = np.asarray(x, np.float32).reshape(B, C, HW)
    in_maps = []
    for core in range(NCORES):
        b, qc = divmod(core, NCORES // B)
        xb = np.roll(xf[b], -qc * QN, axis=1)          # [C, HW]
        xt = xb.reshape(CT, P, 4, QN).transpose(0, 2, 1, 3)
        m = dict(shared)
        m["xb"] = np.ascontiguousarray(xt).astype(ml_dtypes.bfloat16)
        in_maps.append(m)
    return in_maps


def assemble_output(results):
    out = np.empty((B, C, HW), np.float32)
    for core in range(NCORES):
        b, qc = divmod(core, NCORES // B)
        y = np.asarray(results[core]["y"]).reshape(C, QN)
        out[b, :, qc * QN:(qc + 1) * QN] = y
    return out.reshape(B, C, H, W)


def kernel(x, gamma, beta, wq, bq, wk, bk, wv, bv, wp, bp):
    nc = _get_nc()
    in_maps = make_in_maps(x, gamma, beta, wq, bq, wk, bk, wv, bv, wp, bp)
    res = run_bass_kernel_spmd(nc, in_maps, list(range(NCORES)))
    return assemble_output(res.results)


# revision 18
# speedup vs baseline: 1.1968x; 1.1968x over previous
"""Trainium2 Bass kernel for nn_AttentionBlock (GroupNorm + single-head spatial
self-attention + residual) on 8 NeuronCores.

Sharding: data-parallel over batch (2) x sequence-parallel over the query
dimension (4 chunks of 1024 of the 4096 spatial tokens). Each core gets the
full image of its batch element, ROTATED so its query chunk sits at token 0
(GroupNorm stats, key/value sets and softmax sums are permutation-invariant
over tokens, so rotation lets all 8 cores run the identical SPMD program).

All matmul operands are bf16 (measured ~7% faster streaming than f32r, and
half the SBUF/DMA traffic). x ships as bf16 and stays RESIDENT in SBUF for
the whole kernel: GroupNorm stats, all projections, and the residual read it
in place -- no second pass over HBM.

Per-core dataflow (channel-major [C on partitions] everywhere except v):
  phase 1: x DMA (bf16, 4MB, 16 quarter-tile transfers). GroupNorm stats are
           split across three engines so no single engine serializes the
           head: DVE runs bn_stats on token chunks 0-4, ScalarE runs
           Square-with-accumulate (sum of x^2) on chunks 5-7, and the PE
           computes chunk 5-7 per-group sums with a 1/16 block-indicator
           matmul. A second indicator matmul broadcasts group mu/rstd back
           to channels -> per-channel Scale a[c] / Bias b[c].
  fold:    the GroupNorm affine never touches x: the scale a is folded into
           the wq/wk/wv weight tiles (one in-place per-partition multiply on
           each [128, 512] weight tile), and the shift b becomes per-output-
           channel constants computed as ROWS (b^T . wkT = (wk.b)^T, four
           N=512 matmuls per projection) then moved to partition vectors
           with tiny K=1 transpose matmuls; v's constant rides through
           softmax (attention rows sum to 1) and lands in the y epilogue
           constant yb = wp.(wv.b+bv)+bp.
  phase 2: project q/k/v straight out of the resident x tiles:
           k [C, 4096], vT [4096, C] (transposed layout so the later AV
           matmul needs no transposes), q [C, 1024].
  phase 3: per 512-query half: scores^T [j:128, i:512] = k_tile^T @ q (PSUM
           accum over C), exp on ScalarE straight out of PSUM (no max
           subtraction -- logits are O(5)), row-sums r via a ones-vector
           matmul, AV accum hattn[c, i] += vT_tile^T @ p with no transposes.
           Softmax normalization is deferred PAST the output projection
           ((wp.po)/r == wp.(po/r)): unnormalized po is copied to bf16, the
           wp matmuls run immediately, and the 1/r reciprocal + broadcast
           runs in parallel on DVE; the final y = py*rb + (x + yb) is two
           DVE ops against a precomputed x+yb tile. The next half's first
           score/exp iterations are emitted into the tail window so the PE
           never idles across halves.
"""

import sys
from contextlib import ExitStack

if "/opt/trn_rl_repo" not in sys.path:
    sys.path.insert(0, "/opt/trn_rl_repo")

import numpy as np

import concourse.bass as bass  # noqa: F401  (import keeps bass registered)
import concourse.tile as tile
from concourse import bacc, mybir
from concourse.alu_op_type import AluOpType
from concourse.bass_utils import run_bass_kernel_spmd

F32 = mybir.dt.float32
BF16 = mybir.dt.bfloat16
AF = mybir.ActivationFunctionType
OP = AluOpType

B, C, H, W = 2, 512, 64, 64
HW = H * W          # 4096 spatial tokens
P = 128             # partitions
CT = C // P         # 4 channel tiles
NCORES = 8
QN = HW // 4        # 1024 queries per core
CHW = 512           # token chunk width
NCH = HW // CHW     # 8 chunks
JT = HW // P        # 32 key tiles
EPS = 1e-6
SCALE = float(C) ** -0.5
GPT = P // 16       # 8 groups per channel tile
NCHA = 5            # chunks 0..4 on the DVE bn_stats path
NA = NCHA * CHW     # 2560 tokens (path A)
NB = HW - NA        # 1536 tokens (path B: ScalarE sumsq + PE sums)


def _build_body(nc, tc, ctx, d):
    xb_d = d["xb"]
    wT_d = {n: d[n] for n in ("wqT", "wkT", "wvT", "wpT")}
    y_d = d["y"]

    cpool = ctx.enter_context(tc.tile_pool(name="const", bufs=1))
    ppool = ctx.enter_context(tc.tile_pool(name="persist", bufs=1))
    spool = ctx.enter_context(tc.tile_pool(name="stream", bufs=2))
    smpool = ctx.enter_context(tc.tile_pool(name="small", bufs=1))
    qpool = ctx.enter_context(tc.tile_pool(name="psum", bufs=3, space="PSUM"))

    dma_engines = [nc.gpsimd, nc.scalar, nc.sync]

    # ---- phase 1: x arrives bf16 (quarter-tile transfers, earliest tokens
    # first) and stays resident for the whole kernel ----
    x_sb = [ppool.tile([P, HW], BF16, tag=f"x{t}", name=f"x{t}")
            for t in range(CT)]
    for qtr in range(4):
        for t in range(CT):
            eng = dma_engines[(qtr * CT + t) % 3]
            eng.dma_start(x_sb[t][:, qtr * QN:(qtr + 1) * QN], xb_d[t, qtr])
    # small constants right behind the x stream
    ind = cpool.tile([P, GPT], F32, tag="ind")
    nc.gpsimd.dma_start(ind[:], d["ind"][:])
    indb = cpool.tile([P, GPT], BF16, tag="indb")
    nc.scalar.dma_start(indb[:], d["indb"][:])
    indT = cpool.tile([GPT, P], F32, tag="indT")
    nc.sync.dma_start(indT[:], d["indT"][:])
    chv = []
    for t in range(CT):
        v = cpool.tile([P, 6], F32, tag=f"chv{t}", name=f"chv{t}")
        nc.gpsimd.dma_start(v[:], d["chv"][t])
        chv.append(v)
    # chv columns: gamma, beta, bq, bk, bv, bp
    # bulk weights, in consumption order, spread across the 3 DMA queues
    wts = {}
    for wi, name in enumerate(("wkT", "wvT", "wqT", "wpT")):
        wts[name] = []
        for t in range(CT):
            w = cpool.tile([P, C], BF16, tag=f"{name}{t}", name=f"{name}{t}")
            dma_engines[(wi * CT + t) % 3].dma_start(w[:], wT_d[name][t])
            wts[name].append(w)

    ones_col = cpool.tile([P, 1], BF16, tag="onesc")
    nc.vector.memset(ones_col[:], 1.0)
    ones_row = smpool.tile([1, P], BF16, tag="onesr")
    nc.vector.memset(ones_row[:], 1.0)
    one_one = smpool.tile([1, 2], F32, tag="one1")
    nc.vector.memset(one_one[:], 1.0)
    epst = smpool.tile([GPT, CT], F32, tag="eps")
    nc.vector.memset(epst[:], EPS)

    # GroupNorm stats: DVE bn_stats over all 8 chunks
    sts = [smpool.tile([P, NCH, 6], F32, tag="st", bufs=CT, name=f"st{t}")
           for t in range(CT)]
    for ch in range(NCH):
        for t in range(CT):
            nc.vector.bn_stats(sts[t][:, ch, :],
                               x_sb[t][:, ch * CHW:(ch + 1) * CHW])

    gps = qpool.tile([GPT, 2 * CT], F32, tag="pa")
    for t in range(CT):
        mv = smpool.tile([P, 2], F32, tag="mv", bufs=1)
        nc.vector.bn_aggr(mv[:], sts[t][:])
        s3 = smpool.tile([P, 2], F32, tag="s3", bufs=1)
        nc.vector.tensor_copy(s3[:, 0:1], mv[:, 0:1])
        sq = smpool.tile([P, 1], F32, tag="sq", bufs=1)
        nc.vector.tensor_tensor(sq[:], mv[:, 0:1], mv[:, 0:1], op=OP.mult)
        nc.vector.tensor_tensor(s3[:, 1:2], sq[:], mv[:, 1:2], op=OP.add)
        nc.tensor.matmul(gps[:, 2 * t:2 * t + 2], ind[:], s3[:],
                         start=True, stop=True)

    gst = smpool.tile([GPT, 2 * CT], F32, tag="gst")
    nc.vector.tensor_copy(gst[:], gps[:])
    g3 = gst.rearrange("p (t two) -> p t two", two=2)
    mu = smpool.tile([GPT, CT], F32, tag="mu")
    nc.vector.tensor_copy(mu[:], g3[:, :, 0])
    msq = smpool.tile([GPT, CT], F32, tag="msq")
    nc.vector.tensor_tensor(msq[:], mu[:], mu[:], op=OP.mult)
    varg = smpool.tile([GPT, CT], F32, tag="varg")
    nc.vector.tensor_tensor(varg[:], g3[:, :, 1], msq[:], op=OP.subtract)
    stdg = smpool.tile([GPT, CT], F32, tag="stdg")
    nc.scalar.activation(stdg[:], varg[:], AF.Sqrt, bias=epst[:, 0:1])
    # interleave (mu_t, rstd_t) columns and broadcast all groups->channels
    # with a single [K=8, M=128, N=8] indicator matmul
    mr = smpool.tile([GPT, 2 * CT], F32, tag="mr")
    mr3 = mr.rearrange("p (t two) -> p t two", two=2)
    nc.vector.tensor_copy(mr3[:, :, 0], mu[:])
    nc.vector.reciprocal(mr3[:, :, 1], stdg[:])
    cba = qpool.tile([P, 2 * CT], F32, tag="pa")
    nc.tensor.matmul(cba[:], indT[:], mr[:], start=True, stop=True)
    cb = smpool.tile([P, 2 * CT], F32, tag="cb")
    nc.vector.tensor_copy(cb[:], cba[:])

    # per-channel Scale a (col 0) / Bias b (col 1); bvec = bf16 copy of b
    sbts, bvec = [], []
    for t in range(CT):
        sbt = ppool.tile([P, 2], F32, tag=f"sb{t}")
        nc.vector.tensor_tensor(sbt[:, 0:1], cb[:, 2 * t + 1:2 * t + 2],
                                chv[t][:, 0:1], op=OP.mult)
        tmpb = smpool.tile([P, 1], F32, tag="tmpb", bufs=1)
        nc.vector.tensor_tensor(tmpb[:], cb[:, 2 * t:2 * t + 1], sbt[:, 0:1],
                                op=OP.mult)
        nc.vector.tensor_tensor(sbt[:, 1:2], chv[t][:, 1:2], tmpb[:],
                                op=OP.subtract)
        bv_ = ppool.tile([P, 2], BF16, tag=f"bvec{t}", name=f"bvec{t}")
        nc.vector.tensor_copy(bv_[:, 0:1], sbt[:, 1:2])
        nc.vector.tensor_copy(bv_[:, 1:2], sbt[:, 1:2])
        sbts.append(sbt)
        bvec.append(bv_)

    # GroupNorm scale folded into SCALED COPIES of the projection weights
    # (the raw tiles stay live for the bias contracts interleaved into
    # phase 2 below)
    ws = {}
    for name in ("wkT", "wvT", "wqT"):
        ws[name] = []
        for t in range(CT):
            w = cpool.tile([P, C], BF16, tag=f"s{name}{t}", name=f"s{name}{t}")
            nc.vector.tensor_scalar_mul(w[:], wts[name][t][:],
                                        sbts[t][:, 0:1])
            ws[name].append(w)
    ws["wpT"] = wts["wpT"]

    # ---- bias-term constants from RAW weights (tiny N=2 matmuls);
    # emitted one output-tile group at a time, interleaved into phase 2's
    # first chunks so they never head-of-line block the projections ----
    #   qb[o] = sum_c wq[o,c] b[c] + bq    (added at the q PSUM->SBUF move)
    #   kb[o] = likewise with bk
    #   vbt[c] = sum_cin wv[c,cin] b[cin] + bv   (rides softmax into yb)
    #   yb[o] = sum_c wp[o,c] vbt[c] + bp        (y epilogue constant)
    def bias_ct(wname, ot, rhs_tiles, outdt, addcol, tagp):
        pb = qpool.tile([P, 2], F32, tag="pa", name="pb")
        for t in range(CT):
            nc.tensor.matmul(pb[:], wts[wname][t][:, ot * P:(ot + 1) * P],
                             rhs_tiles[t][:, 0:2], start=(t == 0),
                             stop=(t == CT - 1))
        w = 2 if outdt == BF16 else 1
        ob = ppool.tile([P, w], outdt, tag=f"{tagp}{ot}", name=f"{tagp}{ot}")
        if outdt == F32:
            nc.vector.tensor_scalar(ob[:], pb[:, 0:1],
                                    chv[ot][:, addcol:addcol + 1],
                                    None, OP.add)
        else:
            tf = smpool.tile([P, 1], F32, tag="tf", bufs=2)
            nc.vector.tensor_scalar(tf[:], pb[:, 0:1],
                                    chv[ot][:, addcol:addcol + 1],
                                    None, OP.add)
            nc.vector.tensor_copy(ob[:, 0:1], tf[:])
            nc.vector.tensor_copy(ob[:, 1:2], tf[:])
        return ob

    # ---- persistent attention operands (all bf16) ----
    k_sb = [ppool.tile([P, HW], BF16, tag=f"k{t}", name=f"k{t}")
            for t in range(CT)]
    q_sb = [ppool.tile([P, QN], BF16, tag=f"q{t}", name=f"q{t}")
            for t in range(CT)]
    vT_sb = [ppool.tile([P, C], BF16, tag=f"vT{j}", name=f"vT{j}")
             for j in range(JT)]

    # ---- phase 2: q/k/v projections straight from resident x; the bias
    # contracts ride along inside chunks 0-1 (their DVE adds wait, the
    # projection matmuls never do) ----
    kb, qb, vbt, yb = [None] * CT, [None] * CT, [None] * CT, [None] * CT
    for ch in range(NCH):
        sl = slice(ch * CHW, (ch + 1) * CHW)
        for ot in range(CT):
            pk = qpool.tile([P, CHW], F32, tag="pa")
            for t in range(CT):
                nc.tensor.matmul(pk[:], ws["wkT"][t][:, ot * P:(ot + 1) * P],
                                 x_sb[t][:, sl], start=(t == 0),
                                 stop=(t == CT - 1))
            if ch == 0:
                kb[ot] = bias_ct("wkT", ot, bvec, F32, 3, "kb")
            nc.vector.tensor_scalar(k_sb[ot][:, sl], pk[:], kb[ot][:, 0:1],
                                    None, OP.add)
        for nt in range(CT):
            pv = qpool.tile([P, CHW], F32, tag="pa")
            for t in range(CT):
                nc.tensor.matmul(pv[:], x_sb[t][:, ch * CHW + nt * P:
                                                 ch * CHW + (nt + 1) * P],
                                 ws["wvT"][t][:], start=(t == 0),
                                 stop=(t == CT - 1))
            if ch == 0:
                vbt[nt] = bias_ct("wvT", nt, bvec, BF16, 4, "vbt")
            elif ch == 1:
                yb[nt] = bias_ct("wpT", nt, vbt, F32, 5, "yb")
            nc.scalar.copy(vT_sb[ch * CT + nt][:], pv[:])
        if ch * CHW < QN:
            for ot in range(CT):
                pq = qpool.tile([P, CHW], F32, tag="pa")
                for t in range(CT):
                    nc.tensor.matmul(pq[:],
                                     ws["wqT"][t][:, ot * P:(ot + 1) * P],
                                     x_sb[t][:, sl], start=(t == 0),
                                     stop=(t == CT - 1))
                if ch == 0:
                    qb[ot] = bias_ct("wqT", ot, bvec, F32, 2, "qb")
                nc.vector.tensor_scalar(q_sb[ot][:, sl], pq[:], qb[ot][:, 0:1],
                                        None, OP.add)

    # x + yb, precomputed off the critical path for the y epilogue
    xyb = [[None] * CT for _ in range(2)]
    for ih in range(2):
        for ot in range(CT):
            xt = ppool.tile([P, CHW], F32, tag=f"xyb{ih}{ot}",
                            name=f"xyb{ih}{ot}")
            nc.vector.tensor_scalar(xt[:],
                                    x_sb[ot][:, ih * CHW:(ih + 1) * CHW],
                                    yb[ot][:, 0:1], None, OP.add)
            xyb[ih][ot] = xt

    # ---- phase 3: attention, per query half ----
    def mk_pr():
        return qpool.tile([1, CHW], F32, tag="pr", bufs=1, name="pr")

    def mk_po():
        return [qpool.tile([P, CHW], F32, tag=f"po{t}", name=f"po{t}", bufs=1)
                for t in range(CT)]

    def sc_exp(ih, j):
        isl = slice(ih * CHW, (ih + 1) * CHW)
        ps_ = qpool.tile([P, CHW], F32, tag="pa", name="ps")
        for t in range(CT):
            nc.tensor.matmul(ps_[:], k_sb[t][:, j * P:(j + 1) * P],
                             q_sb[t][:, isl], start=(t == 0),
                             stop=(t == CT - 1))
        pT = spool.tile([P, CHW], BF16, tag="pT", bufs=8, name="pT")
        nc.scalar.activation(pT[:], ps_[:], AF.Exp, scale=SCALE)
        return pT

    def rs_av(pr, po, j, pT):
        nc.tensor.matmul(pr[:], ones_col[:], pT[:],
                         start=(j == 0), stop=(j == JT - 1))
        for t in range(CT):
            nc.tensor.matmul(po[t][:], vT_sb[j][:, t * P:(t + 1) * P],
                             pT[:], start=(j == 0), stop=(j == JT - 1))

    def tail_and_y(pr, po, ih):
        isl = slice(ih * CHW, (ih + 1) * CHW)
        # unnormalized h -> bf16 first so the wp matmuls start immediately
        # ((wp.po)/r == wp.(po/r)); copies split across ScalarE and DVE
        has = []
        for t in range(CT):
            ha = spool.tile([P, CHW], BF16, tag=f"hx{t}", bufs=2)
            if t < 2:
                nc.scalar.copy(ha[:], po[t][:])
            else:
                nc.vector.tensor_copy(ha[:], po[t][:])
            has.append(ha)
        # 1/r on DVE (slow iterative reciprocal, but it runs in parallel
        # with the wp matmuls and -- unlike Ln/Exp on ScalarE -- doesn't
        # thrash the activation table, which would stall the next half's
        # exp stream for 2x 1.3us)
        rsb = smpool.tile([1, CHW], F32, tag="rsb", bufs=2)
        nc.vector.tensor_copy(rsb[:], pr[:])
        rinv = smpool.tile([1, CHW], F32, tag="rinv", bufs=2)
        nc.vector.reciprocal(rinv[:], rsb[:])
        rbb = smpool.tile([1, CHW], BF16, tag="rbb", bufs=2)
        nc.vector.tensor_copy(rbb[:], rinv[:])
        rb = spool.tile([P, CHW], F32, tag="rb", bufs=2)
        for ot in range(CT):
            # reuse the freed po slot: the pa slots stay available for the
            # next half's score pipeline even while the 1/r chain lags
            py = qpool.tile([P, CHW], F32, tag=f"po{ot}", name="py", bufs=1)
            for t in range(CT):
                nc.tensor.matmul(py[:], wts["wpT"][t][:, ot * P:(ot + 1) * P],
                                 has[t][:], start=(t == 0), stop=(t == CT - 1))
            if ot == 0:
                prb = qpool.tile([P, CHW], F32, tag="pa")
                nc.tensor.matmul(prb[:], ones_row[:], rbb[:],
                                 start=True, stop=True)
                nc.vector.tensor_copy(rb[:], prb[:])
            t1 = spool.tile([P, CHW], F32, tag="t1", bufs=2)
            nc.vector.tensor_tensor(t1[:], py[:], rb[:], op=OP.mult)
            yt = spool.tile([P, CHW], F32, tag="yt", bufs=3)
            nc.vector.tensor_tensor(yt[:], t1[:], xyb[ih][ot][:], op=OP.add)
            nc.gpsimd.dma_start(y_d[ot, :, isl], yt[:])

    # scores/exp run DEPTH j-groups ahead of the rowsum+AV that consume the
    # exp output, so the PE never waits on ScalarE; KPRE next-half groups
    # are emitted into the tail window so the PE never idles across halves
    DEPTH = 2
    KPRE = 5
    pr0 = mk_pr()
    po0 = mk_po()
    q0 = []
    for j in range(JT):
        q0.append(sc_exp(0, j))
        if j >= DEPTH:
            rs_av(pr0, po0, j - DEPTH, q0[j - DEPTH])
    pr1 = mk_pr()
    pre = []
    for m in range(KPRE):
        pre.append(sc_exp(1, m))
        if m < DEPTH:
            rs_av(pr0, po0, JT - DEPTH + m, q0[JT - DEPTH + m])
    tail_and_y(pr0, po0, 0)
    po1 = mk_po()
    for j in range(JT):
        jj = j + KPRE
        if jj < JT:
            pre.append(sc_exp(1, jj))
        rs_av(pr1, po1, j, pre[j])
    tail_and_y(pr1, po1, 1)


def build_module():
    nc = bacc.Bacc("TRN2", target_bir_lowering=False, debug=False,
                   num_devices=NCORES)
    d = {
        "xb": nc.dram_tensor("xb", [CT, 4, P, QN], BF16,
                             kind="ExternalInput").ap(),
        "wqT": nc.dram_tensor("wqT", [CT, P, C], BF16, kind="ExternalInput").ap(),
        "wkT": nc.dram_tensor("wkT", [CT, P, C], BF16, kind="ExternalInput").ap(),
        "wvT": nc.dram_tensor("wvT", [CT, P, C], BF16, kind="ExternalInput").ap(),
        "wpT": nc.dram_tensor("wpT", [CT, P, C], BF16, kind="ExternalInput").ap(),
        "chv": nc.dram_tensor("chv", [CT, P, 6], F32, kind="ExternalInput").ap(),
        "ind": nc.dram_tensor("ind", [P, GPT], F32, kind="ExternalInput").ap(),
        "indb": nc.dram_tensor("indb", [P, GPT], BF16,
                               kind="ExternalInput").ap(),
        "indT": nc.dram_tensor("indT", [GPT, P], F32, kind="ExternalInput").ap(),
        "y": nc.dram_tensor("y", [CT, P, QN], F32, kind="ExternalOutput").ap(),
    }
    with tile.TileContext(nc) as tc, ExitStack() as ctx:
        _build_body(nc, tc, ctx, d)
    nc.compile()
    return nc


_CACHE = {}


def _get_nc():
    if "nc" not in _CACHE:
        _CACHE["nc"] = build_module()
    return _CACHE["nc"]


def _shared_inputs(gamma, beta, wq, bq, wk, bk, wv, bv, wp, bp):
    import ml_dtypes

    def wT(w):
        wt = np.ascontiguousarray(np.asarray(w, np.float32).T)
        return wt.reshape(CT, P, C).astype(ml_dtypes.bfloat16)

    ind = np.zeros((P, GPT), np.float32)
    for i in range(P):
        ind[i, i // 16] = 1.0 / 16.0
    indT = np.zeros((GPT, P), np.float32)
    for i in range(P):
        indT[i // 16, i] = 1.0
    chv = np.stack([np.asarray(a, np.float32)
                    for a in (gamma, beta, bq, bk, bv, bp)],
                   axis=1).reshape(CT, P, 6)
    return {
        "wqT": wT(wq), "wkT": wT(wk), "wvT": wT(wv), "wpT": wT(wp),
        "chv": np.ascontiguousarray(chv),
        "ind": ind, "indb": ind.astype(ml_dtypes.bfloat16), "indT": indT,
    }


def make_in_maps(x, gamma, beta, wq, bq, wk, bk, wv, bv, wp, bp):
    import ml_dtypes

    shared = _shared_inputs(gamma, beta, wq, bq, wk, bk, wv, bv, wp, bp)
    xf = np.asarray(x, np.float32).reshape(B, C, HW)
    in_maps = []
    for core in range(NCORES):
        b, qc = divmod(core, NCORES // B)
        xb = np.roll(xf[b], -qc * QN, axis=1)          # [C, HW]
        xt = xb.reshape(CT, P, 4, QN).transpose(0, 2, 1, 3)
        m = dict(shared)
        m["xb"] = np.ascontiguousarray(xt).astype(ml_dtypes.bfloat16)
        in_maps.append(m)
    return in_maps


def assemble_output(results):
    out = np.empty((B, C, HW), np.float32)
    for core in range(NCORES):
        b, qc = divmod(core, NCORES // B)
        y = np.asarray(results[core]["y"]).reshape(C, QN)
        out[b, :, qc * QN:(qc + 1) * QN] = y
    return out.reshape(B, C, H, W)


def kernel(x, gamma, beta, wq, bq, wk, bk, wv, bv, wp, bp):
    nc = _get_nc()
    in_maps = make_in_maps(x, gamma, beta, wq, bq, wk, bk, wv, bv, wp, bp)
    res = run_bass_kernel_spmd(nc, in_maps, list(range(NCORES)))
    return assemble_output(res.results)
